# revision 22
# baseline (speedup 1.0000x reference)
"""GAT layer (PyG GATConv eval, 2 heads x 128, self-loops, ELU) on 8 trn2 cores.

v2 strategy (dst-sharded, per core):
  hpack[N,128] f32 rows (512B): cols 0:64 = h as packed bf16 pairs (host),
  cols 64:66 = a_src logits f32 (device phase A embeds).  ONE dma_gather by
  src id per edge slot fetches h (bf16) + a_src together.
  Phase A: a4 = hT_tile^T @ wa4 (host passes hT bf16, no PE transposes),
  embed a_src into hpack; local-shard a_dst kept in SBUF (h_shT input).
  Phase B: edges grouped by dst block (128 dsts), chunks of 128 slots,
  per-block chunk counts = max over cores (not global max).  Per chunk:
  exm = fused is_equal+mult masks (bf16), gtt += hg^T @ exm (bf16 PE),
  segsum via exm^T @ ones, a_dst per slot via mask-transpose matmuls.
  Finalize: U = GT^T W (bf16), normalize, +bias, exact ELU.
"""
import math
from contextlib import ExitStack

import numpy as np
import ml_dtypes

BF16 = ml_dtypes.bfloat16
HEADS = 2
C = 128
IN = 128
N = 50000
NC_CORES = 8
SH = N // NC_CORES            # 6250 dst nodes per core
NBLK = math.ceil(SH / 128)    # 49 dst blocks per core
SHP = NBLK * 128              # padded shard rows 6272
NTILE = math.ceil(N / 128)    # 391 tiles of full h
NPAD = NTILE * 128            # 50048 padded rows of hpack
LO = 32768                    # int16 gather index split
RL = 48                       # max lo chunks per gather run
RH = 28                       # max hi chunks per gather run
NEG_SLOPE = 0.2

_CACHE = {}


# ----------------------------------------------------------------- host prep
def _wrap16(idx, nchunk):
    """idx [nchunk*128] int16 -> wrapped gather table [128, nchunk*8]."""
    sl = idx.reshape(nchunk * 8, 16)            # [col, p16]
    w = np.broadcast_to(sl.T[None, :, :], (8, 16, nchunk * 8))
    return np.ascontiguousarray(w.reshape(128, nchunk * 8))


def _prep(edge_index):
    src = np.concatenate([edge_index[0], np.arange(N, dtype=np.int64)])
    dst = np.concatenate([edge_index[1], np.arange(N, dtype=np.int64)])
    core = dst // SH
    blk = (dst % SH) // 128
    dloc = (dst % SH) % 128
    half = (src >= LO).astype(np.int64)

    # per (core, block, half) counts -> per-block chunk counts (max over cores)
    cnt = np.zeros((NC_CORES, NBLK, 2), dtype=np.int64)
    np.add.at(cnt, (core, blk, half), 1)
    KL = np.maximum(np.ceil(cnt[:, :, 0] / 128).astype(np.int64).max(0), 0)
    KH = np.maximum(np.ceil(cnt[:, :, 1] / 128).astype(np.int64).max(0), 0)
    offL = np.concatenate([[0], np.cumsum(KL)])   # lo-stream chunk offsets
    offH = np.concatenate([[0], np.cumsum(KH)])
    NL, NH = int(offL[-1]), int(offH[-1])

    # slot assignment: stable sort by (core, blk, half); rank within group
    key = (core * NBLK + blk) * 2 + half
    order = np.argsort(key, kind="stable")
    key_s = key[order]
    sizes = np.bincount(key_s, minlength=NC_CORES * NBLK * 2)
    starts = np.concatenate([[0], np.cumsum(sizes)[:-1]])
    rank = np.arange(len(key_s)) - starts[key_s]
    src_s = src[order]
    dloc_s = dloc[order]
    core_s = key_s // (2 * NBLK)
    blk_s = (key_s // 2) % NBLK
    half_s = key_s % 2

    # global slot position within each core's lo/hi stream
    strm_off = np.where(half_s == 0, offL[blk_s] * 128, offH[blk_s] * 128)
    slot = strm_off + rank

    idxL = np.zeros((NC_CORES, NL * 128), dtype=np.int16)
    idxH = np.zeros((NC_CORES, NH * 128), dtype=np.int16)
    dpjL = np.full((NC_CORES, NL, 128), 999.0, dtype=np.float32)
    dpjH = np.full((NC_CORES, NH, 128), 999.0, dtype=np.float32)

    lo_m = half_s == 0
    idxL[core_s[lo_m], slot[lo_m]] = src_s[lo_m].astype(np.int16)
    idxH[core_s[~lo_m], slot[~lo_m]] = (src_s[~lo_m] - LO).astype(np.int16)
    dpjL[core_s[lo_m], slot[lo_m] // 128, slot[lo_m] % 128] = dloc_s[lo_m]
    dpjH[core_s[~lo_m], slot[~lo_m] // 128, slot[~lo_m] % 128] = dloc_s[~lo_m]

    wlo = np.stack([_wrap16(idxL[c], NL) for c in range(NC_CORES)])
    whi = np.stack([_wrap16(idxH[c], NH) for c in range(NC_CORES)])
    # dpj tables [128 partitions, nchunk] (scalar per partition per chunk)
    dpjL_t = np.ascontiguousarray(dpjL.transpose(0, 2, 1))
    dpjH_t = np.ascontiguousarray(dpjH.transpose(0, 2, 1))
    # djp rows [1, nchunk*128] bf16 for the PE broadcast matmul
    djpL = dpjL.reshape(NC_CORES, NL * 128).astype(BF16)
    djpH = dpjH.reshape(NC_CORES, NH * 128).astype(BF16)

    # gather runs: greedy whole blocks with sum KL<=RL and sum KH<=RH
    runs = []
    b = 0
    while b < NBLK:
        b1 = b + 1
        while b1 < NBLK and (KL[b:b1 + 1].sum() <= RL and KH[b:b1 + 1].sum() <= RH):
            b1 += 1
        runs.append((b, b1))
        b = b1
    params = (tuple(int(k) for k in KL), tuple(int(k) for k in KH),
              tuple(runs))
    return params, wlo, whi, dpjL_t, dpjH_t, djpL, djpH


def _pack_inputs(h_node, W, att_src, att_dst, bias):
    hb = h_node.astype(BF16)                       # [N,128] bf16
    hpack = np.zeros((NPAD, 128), dtype=np.float32)
    hpack[:N, 0:64] = hb.view(np.uint16).reshape(N, 64, 2).view(np.uint32).reshape(N, 64).view(np.float32)
    hT = np.zeros((128, NPAD), dtype=BF16)
    hT[:, :N] = hb.T
    h_shT = np.zeros((NC_CORES, 128, SHP), dtype=BF16)
    for c in range(NC_CORES):
        h_shT[c, :, :SH] = hb[c * SH:(c + 1) * SH].T
    W3 = W.reshape(IN, HEADS, C)
    wa4 = np.stack([
        np.einsum('cho,ho->c', W3, att_src * (np.arange(HEADS)[:, None] == 0)),
        np.einsum('cho,ho->c', W3, att_src * (np.arange(HEADS)[:, None] == 1)),
        np.einsum('cho,ho->c', W3, att_dst * (np.arange(HEADS)[:, None] == 0)),
        np.einsum('cho,ho->c', W3, att_dst * (np.arange(HEADS)[:, None] == 1)),
    ], axis=1).astype(BF16)                        # [128, 4]
    wsb = W.astype(BF16)                           # [128, 256]
    bias2 = bias.reshape(1, HEADS * C).astype(np.float32)
    return hpack, hT, h_shT, wa4, wsb, bias2


# ------------------------------------------------------------ device program
def _build(params):
    import concourse.bacc as bacc
    import concourse.bass as bass
    import concourse.mybir as mybir
    import concourse.tile as tile

    KL, KH, runs = params
    offL = [0]
    for k in KL:
        offL.append(offL[-1] + k)
    offH = [0]
    for k in KH:
        offH.append(offH[-1] + k)
    NL, NH = offL[-1], offH[-1]
    KMAX = max(KL[b] + KH[b] for b in range(NBLK))

    dt = mybir.dt
    op = mybir.AluOpType
    act = mybir.ActivationFunctionType
    P = 128

    nc = bacc.Bacc("TRN2", target_bir_lowering=False, debug=False,
                   num_devices=NC_CORES)
    hpack = nc.dram_tensor("hpack", [NPAD, 128], dt.float32, kind="ExternalInput")
    hT_in = nc.dram_tensor("hT", [128, NPAD], dt.bfloat16, kind="ExternalInput")
    hshT_in = nc.dram_tensor("hshT", [128, SHP], dt.bfloat16, kind="ExternalInput")
    wa4_in = nc.dram_tensor("wa4", [128, 4], dt.bfloat16, kind="ExternalInput")
    wsb_in = nc.dram_tensor("wsb", [128, HEADS * C], dt.bfloat16, kind="ExternalInput")
    bias_in = nc.dram_tensor("bias_in", [1, HEADS * C], dt.float32, kind="ExternalInput")
    wlo_in = nc.dram_tensor("wlo", [P, max(NL, 1) * 8], dt.int16, kind="ExternalInput")
    whi_in = nc.dram_tensor("whi", [P, max(NH, 1) * 8], dt.int16, kind="ExternalInput")
    dpjL_in = nc.dram_tensor("dpjL", [P, max(NL, 1)], dt.float32, kind="ExternalInput")
    dpjH_in = nc.dram_tensor("dpjH", [P, max(NH, 1)], dt.float32, kind="ExternalInput")
    djpL_in = nc.dram_tensor("djpL", [1, max(NL, 1) * 128], dt.bfloat16, kind="ExternalInput")
    djpH_in = nc.dram_tensor("djpH", [1, max(NH, 1) * 128], dt.bfloat16, kind="ExternalInput")
    out_t = nc.dram_tensor("out", [SHP, HEADS * C], dt.float32, kind="ExternalOutput")

    with tile.TileContext(nc) as tc, ExitStack() as ctx:
        const = ctx.enter_context(tc.tile_pool(name="const", bufs=1))

        # ---- constants
        iota_row_f = const.tile([P, P], dt.float32)
        nc.gpsimd.iota(iota_row_f[:], pattern=[[1, P]], base=0,
                       channel_multiplier=0, allow_small_or_imprecise_dtypes=True)
        iota_row = const.tile([P, P], dt.bfloat16)
        nc.vector.tensor_copy(out=iota_row[:], in_=iota_row_f[:])
        iota_col4 = const.tile([P, 1024], dt.float32)
        nc.gpsimd.iota(iota_col4[:], pattern=[[0, 1024]], base=0,
                       channel_multiplier=1, allow_small_or_imprecise_dtypes=True)
        iota_colb = const.tile([P, 1024], dt.bfloat16)
        nc.vector.tensor_copy(out=iota_colb[:], in_=iota_col4[:])
        ones1 = const.tile([1, P], dt.bfloat16)
        nc.gpsimd.memset(ones1[:], 1.0)
        ones_col = const.tile([P, 1], dt.bfloat16)
        nc.gpsimd.memset(ones_col[:], 1.0)
        wa4_sb = const.tile([P, 4], dt.bfloat16)
        nc.sync.dma_start(wa4_sb[:], wa4_in.ap()[:, :])
        wsb = const.tile([P, HEADS * C], dt.bfloat16)
        nc.sync.dma_start(wsb[:], wsb_in.ap()[:, :])
        bias_bc = const.tile([P, HEADS * C], dt.float32)
        nc.sync.dma_start(bias_bc[:], bass.AP(bias_in, 0, [[0, P], [1, HEADS * C]]))
        adst_sb = const.tile([P, NBLK, 2], dt.float32)
        adst_bf = const.tile([P, NBLK, 2], dt.bfloat16)

        # ---- phase B input tables (preload during phase A)
        wloT = const.tile([P, max(NL, 1) * 8], dt.int16)
        nc.sync.dma_start(wloT[:], wlo_in.ap()[:, :])
        whiT = const.tile([P, max(NH, 1) * 8], dt.int16)
        nc.sync.dma_start(whiT[:], whi_in.ap()[:, :])
        dpjL_sb = const.tile([P, max(NL, 1)], dt.float32)
        nc.sync.dma_start(dpjL_sb[:], dpjL_in.ap()[:, :])
        dpjH_sb = const.tile([P, max(NH, 1)], dt.float32)
        nc.sync.dma_start(dpjH_sb[:], dpjH_in.ap()[:, :])

        # ---- phase A: a4 = hT_tile^T @ wa4 for all N; embed a_src into hpack
        ctxA = ExitStack()
        sbA = ctxA.enter_context(tc.tile_pool(name="sbA", bufs=2))
        psA = ctxA.enter_context(tc.tile_pool(name="psA", bufs=3, space="PSUM"))
        st = const.tile([P, NTILE, 4], dt.float32)   # a4 staging, all tiles
        GA = 96                     # h tiles per hT DMA
        t = 0
        while t < NTILE:
            nt = min(GA, NTILE - t)
            ht = sbA.tile([P, GA * 128], dt.bfloat16, tag="ht")
            nc.sync.dma_start(ht[:, :nt * 128],
                              hT_in.ap()[:, t * 128:(t + nt) * 128])
            for g0 in range(0, nt, 4):
                n4 = min(4, nt - g0)
                a4p = psA.tile([P, 4, 4], dt.float32, tag="a4", space="PSUM")
                for g in range(n4):
                    nc.tensor.matmul(out=a4p[:, g, :],
                                     lhsT=ht[:, (g0 + g) * 128:(g0 + g + 1) * 128],
                                     rhs=wa4_sb[:], start=True, stop=True)
                nc.scalar.activation(out=st[:, t + g0:t + g0 + n4, :],
                                      in_=a4p[:, :n4, :], func=act.Copy)
            emb_ap = bass.AP(hpack, t * 128 * 128 + 64,
                             [[128, P], [128 * 128, nt], [1, 2]])
            nc.sync.dma_start(emb_ap, st[:, t:t + nt, 0:2])
            t += nt


        # phase A-bis: local shard a_dst from h_shT
        hts = sbA.tile([P, SHP], dt.bfloat16, tag="hts")
        nc.sync.dma_start(hts[:], hshT_in.ap()[:, :])
        for t4 in range(13):
            nt = min(4, NBLK - t4 * 4)
            if nt <= 0:
                break
            a4p = psA.tile([P, 4, 2], dt.float32, tag="a4b", space="PSUM")
            for g in range(nt):
                nc.tensor.matmul(out=a4p[:, g, :],
                                 lhsT=hts[:, (t4 * 4 + g) * 128:(t4 * 4 + g + 1) * 128],
                                 rhs=wa4_sb[:, 2:4], start=True, stop=True)
            nc.scalar.activation(out=adst_sb[:, t4 * 4:t4 * 4 + nt, :],
                                  in_=a4p[:, :nt, :], func=act.Copy)
        nc.vector.tensor_copy(out=adst_bf[:], in_=adst_sb[:])
        ctxA.close()

        # ---- phase B
        ghL = ctx.enter_context(tc.tile_pool(name="ghL", bufs=2))
        ghH = ctx.enter_context(tc.tile_pool(name="ghH", bufs=2))
        gdj = ctx.enter_context(tc.tile_pool(name="gdj", bufs=3))
        mk = ctx.enter_context(tc.tile_pool(name="mk", bufs=5))
        sm = ctx.enter_context(tc.tile_pool(name="sm", bufs=4))
        fin = ctx.enter_context(tc.tile_pool(name="fin", bufs=3))
        psGT = ctx.enter_context(tc.tile_pool(name="psGT", bufs=2, space="PSUM"))
        psAD = ctx.enter_context(tc.tile_pool(name="psAD", bufs=2, space="PSUM"))
        psSS = ctx.enter_context(tc.tile_pool(name="psSS", bufs=1, space="PSUM"))
        psU = ctx.enter_context(tc.tile_pool(name="psU", bufs=2, space="PSUM"))

        hp_ap = hpack.ap()
        for (b0, b1) in runs:
            nL = offL[b1] - offL[b0]
            nH = offH[b1] - offH[b0]
            hgl = ghL.tile([P, RL, 128], dt.float32, tag="hgl")
            if nL:
                nc.gpsimd.dma_gather(
                    out_ap=hgl[:, :nL, :], in_ap=hp_ap[0:LO, :],
                    idxs_ap=wloT[:, offL[b0] * 8:offL[b1] * 8],
                    num_idxs=nL * P, num_idxs_reg=nL * P,
                    elem_size=128, single_packet=False)
            hgh = ghH.tile([P, RH, 128], dt.float32, tag="hgh")
            if nH:
                nc.gpsimd.dma_gather(
                    out_ap=hgh[:, :nH, :], in_ap=hp_ap[LO:NPAD, :],
                    idxs_ap=whiT[:, offH[b0] * 8:offH[b1] * 8],
                    num_idxs=nH * P, num_idxs_reg=nH * P,
                    elem_size=128, single_packet=False)
            djl = gdj.tile([1, RL * 128], dt.bfloat16, tag="djl")
            if nL:
                nc.sync.dma_start(djl[:, :nL * 128],
                                  djpL_in.ap()[:, offL[b0] * 128:offL[b1] * 128])
            djh = gdj.tile([1, RH * 128], dt.bfloat16, tag="djh")
            if nH:
                nc.sync.dma_start(djh[:, :nH * 128],
                                  djpH_in.ap()[:, offH[b0] * 128:offH[b1] * 128])

            for b in range(b0, b1):
                kl, kh = KL[b], KH[b]
                K = kl + kh
                if K == 0:
                    continue
                # chunk descriptors: (hg tile, col in tile, dpj table, pos)
                chunks = []
                for j in range(kl):
                    chunks.append((hgl, offL[b] - offL[b0] + j, dpjL_sb,
                                   offL[b] + j, djl, (offL[b] - offL[b0] + j)))
                for j in range(kh):
                    chunks.append((hgh, offH[b] - offH[b0] + j, dpjH_sb,
                                   offH[b] + j, djh, (offH[b] - offH[b0] + j)))

                # pass 1: a_dst per slot via mask-transpose matmuls
                adp = psAD.tile([P, KMAX, 2], dt.float32, tag="adp", space="PSUM")
                for g0 in range(0, K, 8):
                    ng = min(8, K - g0)
                    dbc = mk.tile([P, 8 * 128], dt.bfloat16, tag="dbc")
                    # pbc needs contiguous djl cols: lo and hi parts separate
                    done = 0
                    while done < ng:
                        djt, dcol = chunks[g0 + done][4], chunks[g0 + done][5]
                        nrun = 1
                        while (done + nrun < ng
                               and chunks[g0 + done + nrun][4] is djt
                               and chunks[g0 + done + nrun][5] == dcol + nrun):
                            nrun += 1
                        nc.gpsimd.partition_broadcast(
                            dbc[:, done * 128:(done + nrun) * 128],
                            djt[0:1, dcol * 128:(dcol + nrun) * 128])
                        done += nrun
                    mt4 = mk.tile([P, 8 * 128], dt.bfloat16, tag="mt4")
                    nc.vector.tensor_tensor(
                        out=mt4[:, :ng * 128], in0=iota_colb[:, :ng * 128],
                        in1=dbc[:, :ng * 128], op=op.is_equal)
                    for gg in range(ng):
                        nc.tensor.matmul(
                            out=adp[:, g0 + gg, :],
                            lhsT=mt4[:, gg * 128:(gg + 1) * 128],
                            rhs=adst_bf[:, b, :], start=True, stop=True)

                # logits -> ex  [P, K, 2]
                tsum = sm.tile([P, KMAX, 2], dt.float32, tag="tsum")
                if kl:
                    nc.vector.tensor_tensor(
                        out=tsum[:, :kl, :],
                        in0=hgl[:, offL[b] - offL[b0]:offL[b] - offL[b0] + kl, 64:66],
                        in1=adp[:, :kl, :], op=op.add)
                if kh:
                    nc.vector.tensor_tensor(
                        out=tsum[:, kl:K, :],
                        in0=hgh[:, offH[b] - offH[b0]:offH[b] - offH[b0] + kh, 64:66],
                        in1=adp[:, kl:K, :], op=op.add)
                u02 = sm.tile([P, KMAX, 2], dt.float32, tag="u02")
                nc.vector.tensor_scalar(out=u02[:, :K, :], in0=tsum[:, :K, :],
                                        scalar1=NEG_SLOPE, scalar2=None, op0=op.mult)
                lr = sm.tile([P, KMAX, 2], dt.float32, tag="lr")
                nc.vector.tensor_tensor(out=lr[:, :K, :], in0=tsum[:, :K, :],
                                        in1=u02[:, :K, :], op=op.max)
                ex2 = sm.tile([P, KMAX, 2], dt.float32, tag="ex2")
                nc.scalar.activation(out=ex2[:, :K, :], in_=lr[:, :K, :], func=act.Exp)

                # pass 2: masked scatter matmuls
                gtt = psGT.tile([P, HEADS * P], dt.float32, tag="gtt", space="PSUM")
                ss0 = psSS.tile([P, 1], dt.float32, tag="ss0", space="PSUM")
                ss1 = psSS.tile([P, 1], dt.float32, tag="ss1", space="PSUM")
                for k, (hg, col, dpjt, dpos, _, _) in enumerate(chunks):
                    st_, sp_ = k == 0, k == K - 1
                    exm = mk.tile([P, 2 * P], dt.bfloat16, tag="exm")
                    nc.vector.tensor_scalar(
                        out=exm[:, 0:P], in0=iota_row[:],
                        scalar1=dpjt[:, dpos:dpos + 1],
                        scalar2=ex2[:, k, 0:1], op0=op.is_equal, op1=op.mult)
                    nc.vector.tensor_scalar(
                        out=exm[:, P:2 * P], in0=iota_row[:],
                        scalar1=dpjt[:, dpos:dpos + 1],
                        scalar2=ex2[:, k, 1:2], op0=op.is_equal, op1=op.mult)
                    nc.tensor.matmul(out=gtt[:], lhsT=hg[:, col, 0:64].bitcast(dt.bfloat16),
                                     rhs=exm[:], start=st_, stop=sp_)
                    nc.tensor.matmul(out=ss0[:], lhsT=exm[:, 0:P], rhs=ones_col[:],
                                     start=st_, stop=sp_)
                    nc.tensor.matmul(out=ss1[:], lhsT=exm[:, P:2 * P], rhs=ones_col[:],
                                     start=st_, stop=sp_)

                # ---- finalize block b
                rec = fin.tile([P, 2], dt.float32, tag="rec")
                nc.vector.reciprocal(out=rec[:, 0:1], in_=ss0[:])
                nc.vector.reciprocal(out=rec[:, 1:2], in_=ss1[:])
                ob = fin.tile([P, HEADS * C], dt.float32, tag="ob")
                for hd in range(HEADS):
                    gs = fin.tile([P, P], dt.bfloat16, tag="gs")
                    nc.scalar.activation(out=gs[:], in_=gtt[:, hd * P:(hd + 1) * P],
                                         func=act.Copy)
                    u = psU.tile([P, C], dt.float32, tag="u", space="PSUM")
                    nc.tensor.matmul(out=u[:], lhsT=gs[:],
                                     rhs=wsb[:, hd * C:(hd + 1) * C],
                                     start=True, stop=True)
                    o2 = fin.tile([P, C], dt.float32, tag="o2")
                    nc.scalar.activation(out=o2[:], in_=u[:], func=act.Copy,
                                         scale=rec[:, hd:hd + 1])
                    o3 = fin.tile([P, C], dt.float32, tag="o3")
                    nc.vector.tensor_tensor(out=o3[:], in0=o2[:],
                                            in1=bias_bc[:, hd * C:(hd + 1) * C],
                                            op=op.add)
                    rl = fin.tile([P, C], dt.float32, tag="rl")
                    nc.scalar.activation(out=rl[:], in_=o3[:], func=act.Relu,
                                         scale=-1.0)
                    e1 = fin.tile([P, C], dt.float32, tag="e1")
                    nc.scalar.activation(out=e1[:], in_=rl[:], func=act.Exp,
                                         scale=-1.0)
                    r2 = fin.tile([P, C], dt.float32, tag="r2")
                    nc.scalar.activation(out=r2[:], in_=o3[:], func=act.Relu)
                    nc.vector.scalar_tensor_tensor(
                        out=ob[:, hd * C:(hd + 1) * C], in0=e1[:], scalar=-1.0,
                        in1=r2[:], op0=op.add, op1=op.add)
                nc.sync.dma_start(out_t.ap()[b * P:(b + 1) * P, :], ob[:])

    nc.compile()
    return nc


def _get_program(params):
    if params not in _CACHE:
        _CACHE[params] = _build(params)
    return _CACHE[params]


# ------------------------------------------------------------------- kernel
def kernel(h_node, edge_index, W, att_src, att_dst, bias):
    from concourse.bass_utils import run_bass_kernel_spmd

    h_node = np.asarray(h_node, dtype=np.float32)
    W = np.asarray(W, dtype=np.float32)
    att_src = np.asarray(att_src, dtype=np.float32)
    att_dst = np.asarray(att_dst, dtype=np.float32)
    bias = np.asarray(bias, dtype=np.float32)

    params, wlo, whi, dpjL_t, dpjH_t, djpL, djpH = _prep(np.asarray(edge_index))
    hpack, hT, h_shT, wa4, wsb, bias2 = _pack_inputs(h_node, W, att_src, att_dst, bias)
    nc = _get_program(params)

    KLs, KHs, _ = params
    NL = sum(KLs)
    NH = sum(KHs)
    in_maps = []
    for c in range(NC_CORES):
        in_maps.append({
            "hpack": hpack, "hT": hT, "hshT": h_shT[c], "wa4": wa4,
            "wsb": wsb, "bias_in": bias2,
            "wlo": wlo[c], "whi": whi[c],
            "dpjL": dpjL_t[c], "dpjH": dpjH_t[c],
            "djpL": djpL[c].reshape(1, NL * 128),
            "djpH": djpH[c].reshape(1, NH * 128),
        })
    res = run_bass_kernel_spmd(nc, in_maps, core_ids=list(range(NC_CORES)))
    out = np.concatenate([res.results[c]["out"][:SH] for c in range(NC_CORES)], axis=0)
    return out


# revision 23
# speedup vs baseline: 1.0009x; 1.0009x over previous
"""GAT layer (PyG GATConv eval, 2 heads x 128, self-loops, ELU) on 8 trn2 cores.

v2 strategy (dst-sharded, per core):
  hpack[N,128] f32 rows (512B): cols 0:64 = h as packed bf16 pairs (host),
  cols 64:66 = a_src logits f32 (device phase A embeds).  ONE dma_gather by
  src id per edge slot fetches h (bf16) + a_src together.
  Phase A: a4 = hT_tile^T @ wa4 (host passes hT bf16, no PE transposes),
  embed a_src into hpack; local-shard a_dst kept in SBUF (h_shT input).
  Phase B: edges grouped by dst block (128 dsts), chunks of 128 slots,
  per-block chunk counts = max over cores (not global max).  Per chunk:
  exm = fused is_equal+mult masks (bf16), gtt += hg^T @ exm (bf16 PE),
  segsum via exm^T @ ones, a_dst per slot via mask-transpose matmuls.
  Finalize: U = GT^T W (bf16), normalize, +bias, exact ELU.
"""
import math
from contextlib import ExitStack

import numpy as np
import ml_dtypes

BF16 = ml_dtypes.bfloat16
HEADS = 2
C = 128
IN = 128
N = 50000
NC_CORES = 8
SH = N // NC_CORES            # 6250 dst nodes per core
NBLK = math.ceil(SH / 128)    # 49 dst blocks per core
SHP = NBLK * 128              # padded shard rows 6272
NTILE = math.ceil(N / 128)    # 391 tiles of full h
NPAD = NTILE * 128            # 50048 padded rows of hpack
LO = 32768                    # int16 gather index split
RL = 48                       # max lo chunks per gather run
RH = 28                       # max hi chunks per gather run
NEG_SLOPE = 0.2

_CACHE = {}


# ----------------------------------------------------------------- host prep
def _wrap16(idx, nchunk):
    """idx [nchunk*128] int16 -> wrapped gather table [128, nchunk*8]."""
    sl = idx.reshape(nchunk * 8, 16)            # [col, p16]
    w = np.broadcast_to(sl.T[None, :, :], (8, 16, nchunk * 8))
    return np.ascontiguousarray(w.reshape(128, nchunk * 8))


def _prep(edge_index):
    src = np.concatenate([edge_index[0], np.arange(N, dtype=np.int64)])
    dst = np.concatenate([edge_index[1], np.arange(N, dtype=np.int64)])
    core = dst // SH
    blk = (dst % SH) // 128
    dloc = (dst % SH) % 128
    half = (src >= LO).astype(np.int64)

    # per (core, block, half) counts -> per-block chunk counts (max over cores)
    cnt = np.zeros((NC_CORES, NBLK, 2), dtype=np.int64)
    np.add.at(cnt, (core, blk, half), 1)
    KL = np.maximum(np.ceil(cnt[:, :, 0] / 128).astype(np.int64).max(0), 0)
    KH = np.maximum(np.ceil(cnt[:, :, 1] / 128).astype(np.int64).max(0), 0)
    offL = np.concatenate([[0], np.cumsum(KL)])   # lo-stream chunk offsets
    offH = np.concatenate([[0], np.cumsum(KH)])
    NL, NH = int(offL[-1]), int(offH[-1])

    # slot assignment: stable sort by (core, blk, half); rank within group
    key = (core * NBLK + blk) * 2 + half
    order = np.argsort(key, kind="stable")
    key_s = key[order]
    sizes = np.bincount(key_s, minlength=NC_CORES * NBLK * 2)
    starts = np.concatenate([[0], np.cumsum(sizes)[:-1]])
    rank = np.arange(len(key_s)) - starts[key_s]
    src_s = src[order]
    dloc_s = dloc[order]
    core_s = key_s // (2 * NBLK)
    blk_s = (key_s // 2) % NBLK
    half_s = key_s % 2

    # global slot position within each core's lo/hi stream
    strm_off = np.where(half_s == 0, offL[blk_s] * 128, offH[blk_s] * 128)
    slot = strm_off + rank

    idxL = np.zeros((NC_CORES, NL * 128), dtype=np.int16)
    idxH = np.zeros((NC_CORES, NH * 128), dtype=np.int16)
    dpjL = np.full((NC_CORES, NL, 128), 999.0, dtype=np.float32)
    dpjH = np.full((NC_CORES, NH, 128), 999.0, dtype=np.float32)

    lo_m = half_s == 0
    idxL[core_s[lo_m], slot[lo_m]] = src_s[lo_m].astype(np.int16)
    idxH[core_s[~lo_m], slot[~lo_m]] = (src_s[~lo_m] - LO).astype(np.int16)
    dpjL[core_s[lo_m], slot[lo_m] // 128, slot[lo_m] % 128] = dloc_s[lo_m]
    dpjH[core_s[~lo_m], slot[~lo_m] // 128, slot[~lo_m] % 128] = dloc_s[~lo_m]

    wlo = np.stack([_wrap16(idxL[c], NL) for c in range(NC_CORES)])
    whi = np.stack([_wrap16(idxH[c], NH) for c in range(NC_CORES)])
    # dpj tables [128 partitions, nchunk] (scalar per partition per chunk)
    dpjL_t = np.ascontiguousarray(dpjL.transpose(0, 2, 1))
    dpjH_t = np.ascontiguousarray(dpjH.transpose(0, 2, 1))
    # djp rows [1, nchunk*128] bf16 for the PE broadcast matmul
    djpL = dpjL.reshape(NC_CORES, NL * 128).astype(BF16)
    djpH = dpjH.reshape(NC_CORES, NH * 128).astype(BF16)

    # gather runs: greedy whole blocks with sum KL<=RL and sum KH<=RH
    runs = []
    b = 0
    while b < NBLK:
        b1 = b + 1
        while b1 < NBLK and (KL[b:b1 + 1].sum() <= RL and KH[b:b1 + 1].sum() <= RH):
            b1 += 1
        runs.append((b, b1))
        b = b1
    params = (tuple(int(k) for k in KL), tuple(int(k) for k in KH),
              tuple(runs))
    return params, wlo, whi, dpjL_t, dpjH_t, djpL, djpH


def _pack_inputs(h_node, W, att_src, att_dst, bias):
    hb = h_node.astype(BF16)                       # [N,128] bf16
    hpack = np.zeros((NPAD, 128), dtype=np.float32)
    hpack[:N, 0:64] = hb.view(np.uint16).reshape(N, 64, 2).view(np.uint32).reshape(N, 64).view(np.float32)
    hT = np.zeros((128, NPAD), dtype=BF16)
    hT[:, :N] = hb.T
    h_shT = np.zeros((NC_CORES, 128, SHP), dtype=BF16)
    for c in range(NC_CORES):
        h_shT[c, :, :SH] = hb[c * SH:(c + 1) * SH].T
    W3 = W.reshape(IN, HEADS, C)
    wa4 = np.stack([
        np.einsum('cho,ho->c', W3, att_src * (np.arange(HEADS)[:, None] == 0)),
        np.einsum('cho,ho->c', W3, att_src * (np.arange(HEADS)[:, None] == 1)),
        np.einsum('cho,ho->c', W3, att_dst * (np.arange(HEADS)[:, None] == 0)),
        np.einsum('cho,ho->c', W3, att_dst * (np.arange(HEADS)[:, None] == 1)),
    ], axis=1).astype(BF16)                        # [128, 4]
    wsb = W.astype(BF16)                           # [128, 256]
    bias2 = bias.reshape(1, HEADS * C).astype(np.float32)
    return hpack, hT, h_shT, wa4, wsb, bias2


# ------------------------------------------------------------ device program
def _build(params):
    import concourse.bacc as bacc
    import concourse.bass as bass
    import concourse.mybir as mybir
    import concourse.tile as tile

    KL, KH, runs = params
    offL = [0]
    for k in KL:
        offL.append(offL[-1] + k)
    offH = [0]
    for k in KH:
        offH.append(offH[-1] + k)
    NL, NH = offL[-1], offH[-1]
    KMAX = max(KL[b] + KH[b] for b in range(NBLK))

    dt = mybir.dt
    op = mybir.AluOpType
    act = mybir.ActivationFunctionType
    P = 128

    nc = bacc.Bacc("TRN2", target_bir_lowering=False, debug=False,
                   num_devices=NC_CORES)
    hpack = nc.dram_tensor("hpack", [NPAD, 128], dt.float32, kind="ExternalInput")
    hT_in = nc.dram_tensor("hT", [128, NPAD], dt.bfloat16, kind="ExternalInput")
    hshT_in = nc.dram_tensor("hshT", [128, SHP], dt.bfloat16, kind="ExternalInput")
    wa4_in = nc.dram_tensor("wa4", [128, 4], dt.bfloat16, kind="ExternalInput")
    wsb_in = nc.dram_tensor("wsb", [128, HEADS * C], dt.bfloat16, kind="ExternalInput")
    bias_in = nc.dram_tensor("bias_in", [1, HEADS * C], dt.float32, kind="ExternalInput")
    wlo_in = nc.dram_tensor("wlo", [P, max(NL, 1) * 8], dt.int16, kind="ExternalInput")
    whi_in = nc.dram_tensor("whi", [P, max(NH, 1) * 8], dt.int16, kind="ExternalInput")
    dpjL_in = nc.dram_tensor("dpjL", [P, max(NL, 1)], dt.float32, kind="ExternalInput")
    dpjH_in = nc.dram_tensor("dpjH", [P, max(NH, 1)], dt.float32, kind="ExternalInput")
    djpL_in = nc.dram_tensor("djpL", [1, max(NL, 1) * 128], dt.bfloat16, kind="ExternalInput")
    djpH_in = nc.dram_tensor("djpH", [1, max(NH, 1) * 128], dt.bfloat16, kind="ExternalInput")
    out_t = nc.dram_tensor("out", [SHP, HEADS * C], dt.float32, kind="ExternalOutput")

    with tile.TileContext(nc) as tc, ExitStack() as ctx:
        const = ctx.enter_context(tc.tile_pool(name="const", bufs=1))

        # ---- constants
        iota_row_f = const.tile([P, P], dt.float32)
        nc.gpsimd.iota(iota_row_f[:], pattern=[[1, P]], base=0,
                       channel_multiplier=0, allow_small_or_imprecise_dtypes=True)
        iota_row = const.tile([P, P], dt.bfloat16)
        nc.vector.tensor_copy(out=iota_row[:], in_=iota_row_f[:])
        iota_col4 = const.tile([P, 1024], dt.float32)
        nc.gpsimd.iota(iota_col4[:], pattern=[[0, 1024]], base=0,
                       channel_multiplier=1, allow_small_or_imprecise_dtypes=True)
        iota_colb = const.tile([P, 1024], dt.bfloat16)
        nc.vector.tensor_copy(out=iota_colb[:], in_=iota_col4[:])
        ones1 = const.tile([1, P], dt.bfloat16)
        nc.gpsimd.memset(ones1[:], 1.0)
        ones_col = const.tile([P, 1], dt.bfloat16)
        nc.gpsimd.memset(ones_col[:], 1.0)
        wa4_sb = const.tile([P, 4], dt.bfloat16)
        nc.sync.dma_start(wa4_sb[:], wa4_in.ap()[:, :])
        wsb = const.tile([P, HEADS * C], dt.bfloat16)
        nc.sync.dma_start(wsb[:], wsb_in.ap()[:, :])
        bias_bc = const.tile([P, HEADS * C], dt.float32)
        nc.sync.dma_start(bias_bc[:], bass.AP(bias_in, 0, [[0, P], [1, HEADS * C]]))
        adst_sb = const.tile([P, NBLK, 2], dt.float32)
        adst_bf = const.tile([P, NBLK, 2], dt.bfloat16)

        # ---- phase B input tables (preload during phase A)
        wloT = const.tile([P, max(NL, 1) * 8], dt.int16)
        nc.sync.dma_start(wloT[:], wlo_in.ap()[:, :])
        whiT = const.tile([P, max(NH, 1) * 8], dt.int16)
        nc.sync.dma_start(whiT[:], whi_in.ap()[:, :])
        dpjL_sb = const.tile([P, max(NL, 1)], dt.float32)
        nc.sync.dma_start(dpjL_sb[:], dpjL_in.ap()[:, :])
        dpjH_sb = const.tile([P, max(NH, 1)], dt.float32)
        nc.sync.dma_start(dpjH_sb[:], dpjH_in.ap()[:, :])

        # ---- phase A: a4 = hT_tile^T @ wa4 for all N; embed a_src into hpack
        ctxA = ExitStack()
        sbA = ctxA.enter_context(tc.tile_pool(name="sbA", bufs=3))
        psA = ctxA.enter_context(tc.tile_pool(name="psA", bufs=3, space="PSUM"))
        st = const.tile([P, NTILE, 4], dt.float32)   # a4 staging, all tiles
        GA = 96                     # h tiles per hT DMA
        t = 0
        while t < NTILE:
            nt = min(GA, NTILE - t)
            ht = sbA.tile([P, GA * 128], dt.bfloat16, tag="ht")
            nc.sync.dma_start(ht[:, :nt * 128],
                              hT_in.ap()[:, t * 128:(t + nt) * 128])
            for g0 in range(0, nt, 4):
                n4 = min(4, nt - g0)
                a4p = psA.tile([P, 4, 4], dt.float32, tag="a4", space="PSUM")
                for g in range(n4):
                    nc.tensor.matmul(out=a4p[:, g, :],
                                     lhsT=ht[:, (g0 + g) * 128:(g0 + g + 1) * 128],
                                     rhs=wa4_sb[:], start=True, stop=True)
                nc.scalar.activation(out=st[:, t + g0:t + g0 + n4, :],
                                      in_=a4p[:, :n4, :], func=act.Copy)
            emb_ap = bass.AP(hpack, t * 128 * 128 + 64,
                             [[128, P], [128 * 128, nt], [1, 2]])
            nc.sync.dma_start(emb_ap, st[:, t:t + nt, 0:2])
            t += nt


        # phase A-bis: local shard a_dst from h_shT
        hts = sbA.tile([P, SHP], dt.bfloat16, tag="hts")
        nc.sync.dma_start(hts[:], hshT_in.ap()[:, :])
        for t4 in range(13):
            nt = min(4, NBLK - t4 * 4)
            if nt <= 0:
                break
            a4p = psA.tile([P, 4, 2], dt.float32, tag="a4b", space="PSUM")
            for g in range(nt):
                nc.tensor.matmul(out=a4p[:, g, :],
                                 lhsT=hts[:, (t4 * 4 + g) * 128:(t4 * 4 + g + 1) * 128],
                                 rhs=wa4_sb[:, 2:4], start=True, stop=True)
            nc.scalar.activation(out=adst_sb[:, t4 * 4:t4 * 4 + nt, :],
                                  in_=a4p[:, :nt, :], func=act.Copy)
        nc.vector.tensor_copy(out=adst_bf[:], in_=adst_sb[:])
        ctxA.close()

        # ---- phase B
        ghL = ctx.enter_context(tc.tile_pool(name="ghL", bufs=2))
        ghH = ctx.enter_context(tc.tile_pool(name="ghH", bufs=2))
        gdj = ctx.enter_context(tc.tile_pool(name="gdj", bufs=2))
        mk = ctx.enter_context(tc.tile_pool(name="mk", bufs=4))
        sm = ctx.enter_context(tc.tile_pool(name="sm", bufs=3))
        fin = ctx.enter_context(tc.tile_pool(name="fin", bufs=3))
        psGT = ctx.enter_context(tc.tile_pool(name="psGT", bufs=2, space="PSUM"))
        psAD = ctx.enter_context(tc.tile_pool(name="psAD", bufs=2, space="PSUM"))
        psSS = ctx.enter_context(tc.tile_pool(name="psSS", bufs=1, space="PSUM"))
        psU = ctx.enter_context(tc.tile_pool(name="psU", bufs=2, space="PSUM"))

        hp_ap = hpack.ap()
        for (b0, b1) in runs:
            nL = offL[b1] - offL[b0]
            nH = offH[b1] - offH[b0]
            hgl = ghL.tile([P, RL, 128], dt.float32, tag="hgl")
            if nL:
                nc.gpsimd.dma_gather(
                    out_ap=hgl[:, :nL, :], in_ap=hp_ap[0:LO, :],
                    idxs_ap=wloT[:, offL[b0] * 8:offL[b1] * 8],
                    num_idxs=nL * P, num_idxs_reg=nL * P,
                    elem_size=128, single_packet=False)
            hgh = ghH.tile([P, RH, 128], dt.float32, tag="hgh")
            if nH:
                nc.gpsimd.dma_gather(
                    out_ap=hgh[:, :nH, :], in_ap=hp_ap[LO:NPAD, :],
                    idxs_ap=whiT[:, offH[b0] * 8:offH[b1] * 8],
                    num_idxs=nH * P, num_idxs_reg=nH * P,
                    elem_size=128, single_packet=False)
            djl = gdj.tile([1, RL * 128], dt.bfloat16, tag="djl")
            if nL:
                nc.sync.dma_start(djl[:, :nL * 128],
                                  djpL_in.ap()[:, offL[b0] * 128:offL[b1] * 128])
            djh = gdj.tile([1, RH * 128], dt.bfloat16, tag="djh")
            if nH:
                nc.sync.dma_start(djh[:, :nH * 128],
                                  djpH_in.ap()[:, offH[b0] * 128:offH[b1] * 128])

            for b in range(b0, b1):
                kl, kh = KL[b], KH[b]
                K = kl + kh
                if K == 0:
                    continue
                # chunk descriptors: (hg tile, col in tile, dpj table, pos)
                chunks = []
                for j in range(kl):
                    chunks.append((hgl, offL[b] - offL[b0] + j, dpjL_sb,
                                   offL[b] + j, djl, (offL[b] - offL[b0] + j)))
                for j in range(kh):
                    chunks.append((hgh, offH[b] - offH[b0] + j, dpjH_sb,
                                   offH[b] + j, djh, (offH[b] - offH[b0] + j)))

                # pass 1: a_dst per slot via mask-transpose matmuls
                adp = psAD.tile([P, KMAX, 2], dt.float32, tag="adp", space="PSUM")
                for g0 in range(0, K, 8):
                    ng = min(8, K - g0)
                    dbc = mk.tile([P, 8 * 128], dt.bfloat16, tag="dbc")
                    # pbc needs contiguous djl cols: lo and hi parts separate
                    done = 0
                    while done < ng:
                        djt, dcol = chunks[g0 + done][4], chunks[g0 + done][5]
                        nrun = 1
                        while (done + nrun < ng
                               and chunks[g0 + done + nrun][4] is djt
                               and chunks[g0 + done + nrun][5] == dcol + nrun):
                            nrun += 1
                        nc.gpsimd.partition_broadcast(
                            dbc[:, done * 128:(done + nrun) * 128],
                            djt[0:1, dcol * 128:(dcol + nrun) * 128])
                        done += nrun
                    mt4 = mk.tile([P, 8 * 128], dt.bfloat16, tag="mt4")
                    nc.vector.tensor_tensor(
                        out=mt4[:, :ng * 128], in0=iota_colb[:, :ng * 128],
                        in1=dbc[:, :ng * 128], op=op.is_equal)
                    for gg in range(ng):
                        nc.tensor.matmul(
                            out=adp[:, g0 + gg, :],
                            lhsT=mt4[:, gg * 128:(gg + 1) * 128],
                            rhs=adst_bf[:, b, :], start=True, stop=True)

                # logits -> ex  [P, K, 2]
                tsum = sm.tile([P, KMAX, 2], dt.float32, tag="tsum")
                if kl:
                    nc.vector.tensor_tensor(
                        out=tsum[:, :kl, :],
                        in0=hgl[:, offL[b] - offL[b0]:offL[b] - offL[b0] + kl, 64:66],
                        in1=adp[:, :kl, :], op=op.add)
                if kh:
                    nc.vector.tensor_tensor(
                        out=tsum[:, kl:K, :],
                        in0=hgh[:, offH[b] - offH[b0]:offH[b] - offH[b0] + kh, 64:66],
                        in1=adp[:, kl:K, :], op=op.add)
                u02 = sm.tile([P, KMAX, 2], dt.float32, tag="u02")
                nc.vector.tensor_scalar(out=u02[:, :K, :], in0=tsum[:, :K, :],
                                        scalar1=NEG_SLOPE, scalar2=None, op0=op.mult)
                lr = sm.tile([P, KMAX, 2], dt.float32, tag="lr")
                nc.vector.tensor_tensor(out=lr[:, :K, :], in0=tsum[:, :K, :],
                                        in1=u02[:, :K, :], op=op.max)
                ex2 = sm.tile([P, KMAX, 2], dt.float32, tag="ex2")
                nc.scalar.activation(out=ex2[:, :K, :], in_=lr[:, :K, :], func=act.Exp)

                # pass 2: masked scatter matmuls
                gtt = psGT.tile([P, HEADS * P], dt.float32, tag="gtt", space="PSUM")
                ss0 = psSS.tile([P, 1], dt.float32, tag="ss0", space="PSUM")
                ss1 = psSS.tile([P, 1], dt.float32, tag="ss1", space="PSUM")
                for k, (hg, col, dpjt, dpos, _, _) in enumerate(chunks):
                    st_, sp_ = k == 0, k == K - 1
                    exm = mk.tile([P, 2 * P], dt.bfloat16, tag="exm")
                    nc.vector.tensor_scalar(
                        out=exm[:, 0:P], in0=iota_row[:],
                        scalar1=dpjt[:, dpos:dpos + 1],
                        scalar2=ex2[:, k, 0:1], op0=op.is_equal, op1=op.mult)
                    nc.vector.tensor_scalar(
                        out=exm[:, P:2 * P], in0=iota_row[:],
                        scalar1=dpjt[:, dpos:dpos + 1],
                        scalar2=ex2[:, k, 1:2], op0=op.is_equal, op1=op.mult)
                    nc.tensor.matmul(out=gtt[:], lhsT=hg[:, col, 0:64].bitcast(dt.bfloat16),
                                     rhs=exm[:], start=st_, stop=sp_)
                    nc.tensor.matmul(out=ss0[:], lhsT=exm[:, 0:P], rhs=ones_col[:],
                                     start=st_, stop=sp_)
                    nc.tensor.matmul(out=ss1[:], lhsT=exm[:, P:2 * P], rhs=ones_col[:],
                                     start=st_, stop=sp_)

                # ---- finalize block b
                rec = fin.tile([P, 2], dt.float32, tag="rec")
                nc.vector.reciprocal(out=rec[:, 0:1], in_=ss0[:])
                nc.vector.reciprocal(out=rec[:, 1:2], in_=ss1[:])
                ob = fin.tile([P, HEADS * C], dt.float32, tag="ob")
                for hd in range(HEADS):
                    gs = fin.tile([P, P], dt.bfloat16, tag="gs")
                    nc.scalar.activation(out=gs[:], in_=gtt[:, hd * P:(hd + 1) * P],
                                         func=act.Copy)
                    u = psU.tile([P, C], dt.float32, tag="u", space="PSUM")
                    nc.tensor.matmul(out=u[:], lhsT=gs[:],
                                     rhs=wsb[:, hd * C:(hd + 1) * C],
                                     start=True, stop=True)
                    o2 = fin.tile([P, C], dt.float32, tag="o2")
                    nc.scalar.activation(out=o2[:], in_=u[:], func=act.Copy,
                                         scale=rec[:, hd:hd + 1])
                    o3 = fin.tile([P, C], dt.float32, tag="o3")
                    nc.vector.tensor_tensor(out=o3[:], in0=o2[:],
                                            in1=bias_bc[:, hd * C:(hd + 1) * C],
                                            op=op.add)
                    rl = fin.tile([P, C], dt.float32, tag="rl")
                    nc.scalar.activation(out=rl[:], in_=o3[:], func=act.Relu,
                                         scale=-1.0)
                    e1 = fin.tile([P, C], dt.float32, tag="e1")
                    nc.scalar.activation(out=e1[:], in_=rl[:], func=act.Exp,
                                         scale=-1.0)
                    r2 = fin.tile([P, C], dt.float32, tag="r2")
                    nc.scalar.activation(out=r2[:], in_=o3[:], func=act.Relu)
                    nc.vector.scalar_tensor_tensor(
                        out=ob[:, hd * C:(hd + 1) * C], in0=e1[:], scalar=-1.0,
                        in1=r2[:], op0=op.add, op1=op.add)
                nc.sync.dma_start(out_t.ap()[b * P:(b + 1) * P, :], ob[:])

    nc.compile()
    return nc


def _get_program(params):
    if params not in _CACHE:
        _CACHE[params] = _build(params)
    return _CACHE[params]


# ------------------------------------------------------------------- kernel
def kernel(h_node, edge_index, W, att_src, att_dst, bias):
    from concourse.bass_utils import run_bass_kernel_spmd

    h_node = np.asarray(h_node, dtype=np.float32)
    W = np.asarray(W, dtype=np.float32)
    att_src = np.asarray(att_src, dtype=np.float32)
    att_dst = np.asarray(att_dst, dtype=np.float32)
    bias = np.asarray(bias, dtype=np.float32)

    params, wlo, whi, dpjL_t, dpjH_t, djpL, djpH = _prep(np.asarray(edge_index))
    hpack, hT, h_shT, wa4, wsb, bias2 = _pack_inputs(h_node, W, att_src, att_dst, bias)
    nc = _get_program(params)

    KLs, KHs, _ = params
    NL = sum(KLs)
    NH = sum(KHs)
    in_maps = []
    for c in range(NC_CORES):
        in_maps.append({
            "hpack": hpack, "hT": hT, "hshT": h_shT[c], "wa4": wa4,
            "wsb": wsb, "bias_in": bias2,
            "wlo": wlo[c], "whi": whi[c],
            "dpjL": dpjL_t[c], "dpjH": dpjH_t[c],
            "djpL": djpL[c].reshape(1, NL * 128),
            "djpH": djpH[c].reshape(1, NH * 128),
        })
    res = run_bass_kernel_spmd(nc, in_maps, core_ids=list(range(NC_CORES)))
    out = np.concatenate([res.results[c]["out"][:SH] for c in range(NC_CORES)], axis=0)
    return out
